# revision 1
# baseline (speedup 1.0000x reference)
"""Non-local (spatial self-attention) denoising block on 8 Trainium2 cores.

Reference math (per sample n, with x:[C,HW], D=C/2):
    t = (W_theta @ x + b_theta) / sqrt(D)      [D, HW]   (1/sqrt(D) folded in)
    p = W_phi   @ x + b_phi                    [D, HW]
    S[q,k] = t[:,q] . p[:,k]
    f = softmax_k(S)
    attn = x @ f.T  (i.e. attn[c,q] = sum_k f[q,k] x[c,k])
    out = x + W_fuse @ attn + b_fuse

Device formulation (all matmuls bf16 with fp32 PSUM accumulation):
    Sᵀ[k,q] = p.T @ t  -- keys on partitions so no transposes are needed
    e = exp(Sᵀ)        -- no max subtraction (|S| <= ~8 by construction)
    G'ᵀ = xᵀ @ W_fuseᵀ -- fuses the output conv into the values: [HW, C]
    es = Σ_tiles e     -- running sum on DVE (fp32-exact)
    Zb = onesᵀ @ es    -- Z[q] replicated on all 128 partitions (sum+broadcast)
    y = G'ᵀᵀ @ e       -- [C, HW] unnormalized
    out = y * (1/Zb) + (x + b_fuse)   (division commutes through the conv)

Sharding: data-parallel over batch N=32 -> 4 samples per core on 8 cores.
Scale 1/sqrt(D) and b_fuse are folded host-side; weights are host-transposed
into lhsT layouts; a ~3.4us burst of junk matmuls pre-warms the PE clock
(HAM) while the first DMAs land.
"""

import numpy as np
import ml_dtypes

import concourse.bass as bass
import concourse.tile as tile
from concourse import bacc, mybir
from concourse import bass_utils

F32 = mybir.dt.float32
BF16 = mybir.dt.bfloat16
AF = mybir.ActivationFunctionType

N, C, H, W = 32, 512, 32, 32
D = C // 2
HW = H * W
NCORES = 8
NS = N // NCORES  # samples per core
P = 128
CT = C // P   # 4 c-tiles
KT = HW // P  # 8 hw-tiles
MT_D = (2 * D) // P  # 4 m-tiles of combined theta/phi conv
NQ = HW // 512  # 2 free-dim halves


def _emit(tc):
    nc = tc.nc

    x_bf = nc.dram_tensor("x_bf", [NS, C, HW], BF16, kind="ExternalInput").ap()
    x_res = nc.dram_tensor("x_res", [NS, C, HW], F32, kind="ExternalInput").ap()
    wcat_t = nc.dram_tensor("wcat_t", [C, 2 * D], BF16, kind="ExternalInput").ap()
    b_cat = nc.dram_tensor("b_cat", [2 * D, 1], F32, kind="ExternalInput").ap()
    wfu_t = nc.dram_tensor("wfu_t", [C, C], BF16, kind="ExternalInput").ap()
    out_d = nc.dram_tensor("out", [NS, C, HW], F32, kind="ExternalOutput").ap()

    import contextlib
    ctx = contextlib.ExitStack()
    with ctx:
        # ---- constant pools ----
        wpool = ctx.enter_context(tc.tile_pool(name="wpool", bufs=1))
        # weights: [c, *] chunked by 128 c-rows along free dim.
        # wcat + x(sample 0) gate the first conv matmuls: split them into
        # per-c-chunk DMAs on two queues so matmul k-steps start as chunks land.
        wcat_sb = wpool.tile([P, CT * 2 * D], BF16)

        # ---- working pools ----
        xbf_pool = ctx.enter_context(tc.tile_pool(name="xbf", bufs=3))
        xres_pool = ctx.enter_context(tc.tile_pool(name="xres", bufs=2))
        tp_pool = ctx.enter_context(tc.tile_pool(name="tp", bufs=2))
        gt_pool = ctx.enter_context(tc.tile_pool(name="gt", bufs=2))
        e_pool = ctx.enter_context(tc.tile_pool(name="e", bufs=2))
        rz_pool = ctx.enter_context(tc.tile_pool(name="rz", bufs=2))
        fin_pool = ctx.enter_context(tc.tile_pool(name="fin", bufs=3))
        out_pool = ctx.enter_context(tc.tile_pool(name="outp", bufs=3))

        # one PSUM pool: 4 slots x 2 banks = all 8 banks. The 3-tile slack
        # absorbs ACT's exp lag (1.15us/tile) behind PE's S^T rate (0.86us).
        psum_mm = ctx.enter_context(tc.tile_pool(name="psmm", bufs=4, space="PSUM"))
        esum_pool = ctx.enter_context(tc.tile_pool(name="esum", bufs=1))

        # HAM pre-warm: ~3.4us of junk matmuls during the initial DMA wait
        # so the real matmuls start at 2.4 GHz instead of 1.2 GHz.
        ones_sb = wpool.tile([P, P], F32)
        nc.vector.memset(ones_sb[:], 1.0)
        ones_bf = wpool.tile([P, P], BF16)
        nc.vector.memset(ones_bf[:], 1.0)
        warm_rhs = wpool.tile([P, 512], BF16)
        nc.vector.memset(warm_rhs[:], 0.0)
        # 9 x N=512 cold matmuls ~= 3.8us busy > the 3.4us HAM window, so the
        # clock flips to 2.4GHz before the first real matmul
        ps_warm = psum_mm.tile([P, 512], F32, tag="mm", name="ps_warm")
        for w in range(9):
            nc.tensor.matmul(ps_warm[:], ones_bf[:], warm_rhs[:],
                             start=True, stop=True)

        xbf_tiles = {}
        xbf_tiles[0] = xbf_pool.tile([P, CT * HW], BF16, tag="xbf", name="xbf0")
        for k in range(CT):
            nc.sync.dma_start(
                wcat_sb[:, k * 2 * D:(k + 1) * 2 * D],
                wcat_t.rearrange("(t p) d -> t p d", p=P)[k],
            )
            nc.gpsimd.dma_start(
                xbf_tiles[0][:, k * HW:(k + 1) * HW],
                x_bf[0].rearrange("(t p) f -> t p f", p=P)[k],
            )

        # remaining constants (not needed by the first conv matmuls).
        # wfu rides the gpsimd ring behind the x chunks: the sync ring is
        # backed up with xres/xbf and was delivering wfu after gT started.
        wfu_sb = wpool.tile([P, CT * C], BF16)
        nc.gpsimd.dma_start(
            wfu_sb.rearrange("p (t d) -> p t d", d=C),
            wfu_t.rearrange("(t p) d -> p t d", p=P),
        )
        bcat_sb = wpool.tile([P, MT_D], F32)
        nc.sync.dma_start(
            bcat_sb.rearrange("p (t o) -> p t o", o=1),
            b_cat.rearrange("(t p) o -> p t o", p=P),
        )
        for s in range(NS):
            # ---- load x (bf16 for matmuls, f32 residual w/ b_fuse folded) ----
            if s not in xbf_tiles:
                xbf_tiles[s] = xbf_pool.tile(
                    [P, CT * HW], BF16, tag="xbf", name=f"xbf{s}"
                )
                nc.sync.dma_start(
                    xbf_tiles[s].rearrange("p (t f) -> p t f", f=HW),
                    x_bf[s].rearrange("(t p) f -> p t f", p=P),
                )
            xbf_sb = xbf_tiles[s]
            xres_sb = xres_pool.tile([P, CT * HW], F32, tag="xres")
            nc.sync.dma_start(
                xres_sb.rearrange("p (t f) -> p t f", f=HW),
                x_res[s].rearrange("(t p) f -> p t f", p=P),
            )

            # ---- combined theta/phi 1x1 conv: tp = wcat.T @ x + b ----
            # tp_sb chunks m=0,1 -> theta [256, HW]; m=2,3 -> phi
            # Sample 0 runs k-outer/m-inner (4 live psum tiles = whole pool)
            # so each arriving x-chunk feeds 8 matmuls immediately instead of
            # stalling the m0 accumulation on chunks still in flight.
            tp_sb = tp_pool.tile([P, MT_D * HW], BF16, tag="tp")

            def conv_mm(ps, m, k):
                for nq in range(NQ):
                    nc.tensor.matmul(
                        ps[:, nq * 512:(nq + 1) * 512],
                        wcat_sb[:, k * 2 * D + m * P: k * 2 * D + (m + 1) * P],
                        xbf_sb[:, k * HW + nq * 512: k * HW + nq * 512 + 512],
                        start=(k == 0),
                        stop=(k == CT - 1),
                    )

            def conv_copy(ps, m):
                # on DVE (tensor_scalar add w/ per-partition bias): keeps ACT
                # free for exps, so PSUM slots hand off without backlog stalls
                nc.vector.tensor_scalar_add(
                    tp_sb[:, m * HW:(m + 1) * HW], ps[:],
                    bcat_sb[:, m:m + 1],
                )

            if s == 0:
                ps_cvs = [
                    psum_mm.tile([P, HW], F32, tag="mm", name=f"ps_cv0_{m}")
                    for m in range(MT_D)
                ]
                for k in range(CT):
                    for m in range(MT_D):
                        conv_mm(ps_cvs[m], m, k)
                for m in range(MT_D):
                    conv_copy(ps_cvs[m], m)
            else:
                for m in range(MT_D):
                    ps_cv = psum_mm.tile(
                        [P, HW], F32, tag="mm", name=f"ps_cv{s}_{m}"
                    )
                    for k in range(CT):
                        conv_mm(ps_cv, m, k)
                    conv_copy(ps_cv, m)

            # ---- G'T = x.T @ wfu.T : [HW, C], fused-values ----
            # For sample 0 this phase is emitted AFTER S^T: G'T is only
            # needed by the y matmuls, and delaying it gives the wfu DMA
            # ~14us of compute cover instead of ~7us (both rings are still
            # draining x chunks at the head).
            gt_sb = gt_pool.tile([P, KT * C], BF16, tag="gt")

            def gt_phase():
                for m in range(KT):
                    ps_g = psum_mm.tile(
                        [P, C], F32, tag="mm", name=f"ps_g{s}_{m}"
                    )
                    for k in range(CT):
                        nc.tensor.matmul(
                            ps_g[:],
                            xbf_sb[:, k * HW + m * P: k * HW + (m + 1) * P],
                            wfu_sb[:, k * C:(k + 1) * C],
                            start=(k == 0),
                            stop=(k == CT - 1),
                        )
                    nc.scalar.activation(
                        gt_sb[:, m * C:(m + 1) * C], ps_g[:], AF.Copy,
                    )

            gt_phase()

            # ---- S^T = p.T @ t ; e = exp(S^T) ----
            # Z via add-tree over the 8 e-tiles (engine ALUs are fp32-internal,
            # so f32 outputs make the partial sums exact), then one ones-matmul
            # both sums over the 128 partitions and broadcasts Z to them.
            e_sb = e_pool.tile([P, KT * HW], BF16, tag="e")
            es_sb = esum_pool.tile([P, HW], F32, tag="es")
            for m in range(KT):
                ps_s = psum_mm.tile([P, HW], F32, tag="mm", name=f"ps_s{s}_{m}")
                for kd in range(2):
                    for nq in range(NQ):
                        nc.tensor.matmul(
                            ps_s[:, nq * 512:(nq + 1) * 512],
                            tp_sb[:, (2 + kd) * HW + m * P: (2 + kd) * HW + (m + 1) * P],
                            tp_sb[:, kd * HW + nq * 512: kd * HW + nq * 512 + 512],
                            start=(kd == 0),
                            stop=(kd == 1),
                        )
                nc.scalar.activation(
                    e_sb[:, m * HW:(m + 1) * HW], ps_s[:], AF.Exp,
                )
                # running Z sum on DVE (fp32-internal, so exact): each add
                # needs only the newest e tile -> the chain finishes one add
                # after the last exp instead of a full tree depth after it
                if m == 1:
                    nc.vector.tensor_add(
                        es_sb[:], e_sb[:, 0:HW], e_sb[:, HW:2 * HW],
                    )
                elif m > 1:
                    nc.vector.tensor_add(
                        es_sb[:], es_sb[:], e_sb[:, m * HW:(m + 1) * HW],
                    )

            # ---- y = G'T.T @ e : [C, HW] unnormalized attn+conv ----
            # PE order: y0, y1, Zb, y2, y3 -- Zb depends on the DVE/GpSimd
            # add-tree, so it is placed 2 m-tiles deep to hide the tree
            # latency; mul[0] (ready once rzb is) frees y0's PSUM slot for y3.
            def y_mmtile(m):
                ps_y = psum_mm.tile([P, HW], F32, tag="mm", name=f"ps_y{s}_{m}")
                for k in range(KT):
                    for nq in range(NQ):
                        nc.tensor.matmul(
                            ps_y[:, nq * 512:(nq + 1) * 512],
                            gt_sb[:, k * C + m * P: k * C + (m + 1) * P],
                            e_sb[:, k * HW + nq * 512: k * HW + nq * 512 + 512],
                            start=(k == 0),
                            stop=(k == KT - 1),
                        )
                return ps_y

            ps_ys = [y_mmtile(0), y_mmtile(1)]

            # partition-sum + broadcast of Z in one accumulation group
            ps_zbt = psum_mm.tile([P, HW], F32, tag="mm", name=f"ps_zb{s}")
            ps_zb, ps_zb2 = ps_zbt[:, 0:512], ps_zbt[:, 512:HW]
            for nq, pz in enumerate((ps_zb, ps_zb2)):
                nc.tensor.matmul(
                    pz[:],
                    ones_sb[:],
                    es_sb[:, nq * 512:(nq + 1) * 512],
                    start=True,
                    stop=True,
                )
            rzb_sb = rz_pool.tile([P, HW], F32, tag="rz")
            nc.vector.reciprocal_approx_fast(out=rzb_sb[:, 0:512], in_=ps_zb[:])
            nc.vector.reciprocal_approx_fast(out=rzb_sb[:, 512:HW], in_=ps_zb2[:])

            ps_ys += [y_mmtile(2), y_mmtile(3)]

            # final combine in 512-halves: mul (psum, must be DVE) then the
            # residual add on GpSimd (idle otherwise) -- halves the DVE chain
            # that gates the kernel tail
            for m in range(CT):
                t1 = fin_pool.tile([P, HW], F32, tag="fin", name=f"t1_{s}_{m}")
                o_sb = out_pool.tile([P, HW], F32, tag="o", name=f"o_{s}_{m}")
                # the last sample's residual adds go to GpSimd: its tail chain
                # is the kernel tail, and GpSimd is idle there (elsewhere the
                # DVE/GpSimd shared SBUF port makes GpSimd adds a net loss)
                add_eng = nc.gpsimd if s == NS - 1 else nc.vector
                for h in range(2):
                    hs = slice(h * 512, (h + 1) * 512)
                    nc.vector.tensor_mul(t1[:, hs], ps_ys[m][:, hs], rzb_sb[:, hs])
                    add_eng.tensor_add(
                        o_sb[:, hs], t1[:, hs],
                        xres_sb[:, m * HW + h * 512: m * HW + h * 512 + 512],
                    )
                    nc.sync.dma_start(
                        out_d[s].rearrange("(t p) f -> t p f", p=P)[m][:, hs],
                        o_sb[:, hs],
                    )


_CACHE = {}


def _build():
    if "nc" not in _CACHE:
        nc = bacc.Bacc("TRN2", target_bir_lowering=False, debug=False)
        with tile.TileContext(nc) as tc:
            _emit(tc)
        nc.compile()
        _CACHE["nc"] = nc
    return _CACHE["nc"]


def _prep_in_maps(x, W_theta, b_theta, W_phi, b_phi, W_fuse, b_fuse):
    bf = ml_dtypes.bfloat16
    scale = np.float32(1.0 / np.sqrt(np.float32(D)))
    xf = np.ascontiguousarray(x.reshape(N, C, HW).astype(np.float32))
    x_bf = xf.astype(bf)
    x_res = xf + b_fuse.astype(np.float32)[None, :, None]
    wcat_t = np.ascontiguousarray(
        np.concatenate([W_theta.astype(np.float32) * scale,
                        W_phi.astype(np.float32)], axis=0).T
    ).astype(bf)
    b_cat = np.concatenate([b_theta.astype(np.float32) * scale,
                            b_phi.astype(np.float32)]).reshape(2 * D, 1)
    wfu_t = np.ascontiguousarray(W_fuse.astype(np.float32).T).astype(bf)

    in_maps = []
    for c in range(NCORES):
        sl = slice(c * NS, (c + 1) * NS)
        in_maps.append({
            "x_bf": np.ascontiguousarray(x_bf[sl]),
            "x_res": np.ascontiguousarray(x_res[sl]),
            "wcat_t": wcat_t,
            "b_cat": b_cat.astype(np.float32),
            "wfu_t": wfu_t,
        })
    return in_maps


def _run(inputs, trace=False, **kw):
    nc = _build()
    in_maps = _prep_in_maps(**inputs)
    res = bass_utils.run_bass_kernel_spmd(
        nc, in_maps, core_ids=list(range(NCORES)), trace=trace, **kw
    )
    out = np.concatenate([res.results[c]["out"] for c in range(NCORES)], axis=0)
    return out.reshape(N, C, H, W).astype(np.float32), res


def kernel(**inputs):
    inputs = {k: np.asarray(v) for k, v in inputs.items()}
    out, _ = _run(inputs, trace=False)
    return out



# revision 4
# speedup vs baseline: 1.1433x; 1.1433x over previous
"""Non-local (spatial self-attention) denoising block on 8 Trainium2 cores.

Reference math (per sample n, with x:[C,HW], D=C/2):
    t = (W_theta @ x + b_theta) / sqrt(D)      [D, HW]
    p = W_phi   @ x + b_phi                    [D, HW]
    S[q,k] = t[:,q] . p[:,k]
    f = softmax_k(S)
    attn = x @ f.T
    out = x + W_fuse @ attn + b_fuse

Device formulation -- all four matmul phases run fp8e4 (e4m3) DoubleRow,
which packs two 128-row k-tiles per instruction (~1.9x bf16 PE throughput):
    Sᵀ[k,q] = p.T @ t   -- keys on partitions so no transposes are needed
    e = exp(S/BOOST - 3) -- shift is softmax-invariant; keeps e in e4m3 range
    G'ᵀ = xᵀ @ (16·W_fuseᵀ)  -- output conv fused into the values: [HW, C]
    es = Σ_tiles e      -- running sum on DVE (fp32 exact), last add -> bf16
    Zb = (16·ones)ᵀ @ es -- Z on all 128 partitions; 16 cancels the wfu boost
    y = G'ᵀᵀ @ e        -- [C, HW] unnormalized
    out = y * (1/Zb) + (x + b_fuse)

fp8 scale management (e4m3: max 240, min normal 2^-7): W_theta/W_phi are
boosted x8 host-side (raw 0.05-scale weights would sit in subnormals),
W_fuse x16; the 1/sqrt(D) softmax scale and both theta/phi boosts fold into
the exp activation's scale (1/1024), the W_fuse boost into the ones vector.

Sharding: data-parallel over batch N=32 -> 4 samples per core on 8 cores.
A ~3.8us burst of junk matmuls pre-warms the PE clock (HAM) while the first
DMAs land.
"""

import numpy as np
import ml_dtypes

import concourse.bass as bass
import concourse.tile as tile
from concourse import bacc, mybir
from concourse import bass_utils

F32 = mybir.dt.float32
BF16 = mybir.dt.bfloat16
F8 = mybir.dt.float8e4
AF = mybir.ActivationFunctionType
DR = mybir.MatmulPerfMode.DoubleRow

N, C, H, W = 32, 512, 32, 32
D = C // 2
HW = H * W
NCORES = 8
NS = N // NCORES  # samples per core
P = 128
CT = C // P   # 4 c-tiles
KT = HW // P  # 8 hw-tiles
MT_D = (2 * D) // P  # 4 m-tiles of combined theta/phi conv
NQ = HW // 512  # 2 free-dim halves

# host-side power-of-2 boosts to keep fp8 operands out of subnormal range
TP_BOOST = 8.0     # on W_theta and W_phi (and their biases)
FU_BOOST = 16.0    # on W_fuse
EXP_SCALE = 1.0 / (TP_BOOST * TP_BOOST * np.sqrt(np.float32(D)))
EXP_BIAS = -3.0    # softmax-shift: keeps exp() within e4m3 range


def _emit(tc):
    nc = tc.nc

    x_f8 = nc.dram_tensor("x_f8", [NS, C, HW], F8, kind="ExternalInput").ap()
    x_res = nc.dram_tensor("x_res", [NS, C, HW], BF16, kind="ExternalInput").ap()
    wcat_t = nc.dram_tensor("wcat_t", [C, 2 * D], F8, kind="ExternalInput").ap()
    b_cat = nc.dram_tensor("b_cat", [2 * D, 1], F32, kind="ExternalInput").ap()
    wfu_t = nc.dram_tensor("wfu_t", [C, C], F8, kind="ExternalInput").ap()
    out_d = nc.dram_tensor("out", [NS, C, HW], F32, kind="ExternalOutput").ap()

    import contextlib
    ctx = contextlib.ExitStack()
    with ctx:
        # ---- constant pools ----
        wpool = ctx.enter_context(tc.tile_pool(name="wpool", bufs=1))
        # weights as 3D [P, ktile, free] so DoubleRow can slice k-pairs
        wcat_sb = wpool.tile([P, CT, 2 * D], F8)

        # ---- working pools ----
        xf8_pool = ctx.enter_context(tc.tile_pool(name="xf8", bufs=3))
        xres_pool = ctx.enter_context(tc.tile_pool(name="xres", bufs=2))
        tp_pool = ctx.enter_context(tc.tile_pool(name="tp", bufs=2))
        gt_pool = ctx.enter_context(tc.tile_pool(name="gt", bufs=2))
        e_pool = ctx.enter_context(tc.tile_pool(name="e", bufs=2))
        rz_pool = ctx.enter_context(tc.tile_pool(name="rz", bufs=2))
        fin_pool = ctx.enter_context(tc.tile_pool(name="fin", bufs=3))
        out_pool = ctx.enter_context(tc.tile_pool(name="outp", bufs=3))

        psum_mm = ctx.enter_context(tc.tile_pool(name="psmm", bufs=4, space="PSUM"))
        esum_pool = ctx.enter_context(tc.tile_pool(name="esum", bufs=1))

        # HAM pre-warm: ~3.8us of junk matmuls during the initial DMA wait
        # so the real matmuls start at 2.4 GHz instead of 1.2 GHz.
        ones_bf = wpool.tile([P, P], BF16)
        nc.vector.memset(ones_bf[:], FU_BOOST)  # folds the wfu boost out of Z
        warm_rhs = wpool.tile([P, 512], BF16)
        nc.vector.memset(warm_rhs[:], 0.0)
        ps_warm = psum_mm.tile([P, 512], F32, tag="mm", name="ps_warm")
        for w in range(9):
            nc.tensor.matmul(ps_warm[:], ones_bf[:], warm_rhs[:],
                             start=True, stop=True)

        xf8_tiles = {}
        xf8_tiles[0] = xf8_pool.tile([P, CT, HW], F8, tag="xf8", name="xf80")
        for k in range(CT):
            nc.sync.dma_start(
                wcat_sb[:, k, :],
                wcat_t.rearrange("(t p) d -> t p d", p=P)[k],
            )
            nc.gpsimd.dma_start(
                xf8_tiles[0][:, k, :],
                x_f8[0].rearrange("(t p) f -> t p f", p=P)[k],
            )

        # remaining constants (not needed by the first conv matmuls)
        wfu_sb = wpool.tile([P, CT, C], F8)
        nc.gpsimd.dma_start(
            wfu_sb[:],
            wfu_t.rearrange("(t p) d -> p t d", p=P),
        )
        bcat_sb = wpool.tile([P, MT_D], F32)
        nc.sync.dma_start(
            bcat_sb.rearrange("p (t o) -> p t o", o=1),
            b_cat.rearrange("(t p) o -> p t o", p=P),
        )
        ebias_sb = wpool.tile([P, 1], F32)
        nc.vector.memset(ebias_sb[:], EXP_BIAS)
        for s in range(NS):
            # ---- load x (fp8 for matmuls, bf16 residual w/ b_fuse folded) ----
            if s not in xf8_tiles:
                xf8_tiles[s] = xf8_pool.tile(
                    [P, CT, HW], F8, tag="xf8", name=f"xf8{s}"
                )
                nc.sync.dma_start(
                    xf8_tiles[s][:],
                    x_f8[s].rearrange("(t p) f -> p t f", p=P),
                )
            xf8_sb = xf8_tiles[s]
            xres_sb = xres_pool.tile([P, CT, HW], BF16, tag="xres")
            nc.sync.dma_start(
                xres_sb[:],
                x_res[s].rearrange("(t p) f -> p t f", p=P),
            )

            # ---- combined theta/phi 1x1 conv: tp = wcat.T @ x + b ----
            # tp_sb chunks m=0,1 -> theta [256, HW]; m=2,3 -> phi
            tp_sb = tp_pool.tile([P, MT_D, HW], F8, tag="tp")

            def conv_mm(ps, m, kp):
                for nq in range(NQ):
                    nc.tensor.matmul(
                        ps[:, nq * 512:(nq + 1) * 512],
                        wcat_sb[:, 2 * kp:2 * kp + 2, m * P:(m + 1) * P],
                        xf8_sb[:, 2 * kp:2 * kp + 2, nq * 512:nq * 512 + 512],
                        start=(kp == 0),
                        stop=(kp == CT // 2 - 1),
                        perf_mode=DR,
                    )

            def conv_copy(ps, m):
                # on DVE (tensor_scalar add w/ per-partition bias): keeps ACT
                # free for exps
                nc.vector.tensor_scalar_add(
                    tp_sb[:, m, :], ps[:], bcat_sb[:, m:m + 1],
                )

            if s == 0:
                # k-outer so each arriving x k-pair feeds all m immediately
                ps_cvs = [
                    psum_mm.tile([P, HW], F32, tag="mm", name=f"ps_cv0_{m}")
                    for m in range(MT_D)
                ]
                for kp in range(CT // 2):
                    for m in range(MT_D):
                        conv_mm(ps_cvs[m], m, kp)
                for m in range(MT_D):
                    conv_copy(ps_cvs[m], m)
            else:
                for m in range(MT_D):
                    ps_cv = psum_mm.tile(
                        [P, HW], F32, tag="mm", name=f"ps_cv{s}_{m}"
                    )
                    for kp in range(CT // 2):
                        conv_mm(ps_cv, m, kp)
                    conv_copy(ps_cv, m)

            # ---- G'T = x.T @ (16 wfu.T) : [HW, C], fused-values ----
            gt_sb = gt_pool.tile([P, KT, C], F8, tag="gt")

            def gt_mtile(m):
                ps_g = psum_mm.tile([P, C], F32, tag="mm", name=f"ps_g{s}_{m}")
                for kp in range(CT // 2):
                    nc.tensor.matmul(
                        ps_g[:],
                        xf8_sb[:, 2 * kp:2 * kp + 2, m * P:(m + 1) * P],
                        wfu_sb[:, 2 * kp:2 * kp + 2, :],
                        start=(kp == 0),
                        stop=(kp == CT // 2 - 1),
                        perf_mode=DR,
                    )
                nc.scalar.activation(gt_sb[:, m, :], ps_g[:], AF.Copy)

            # ---- S^T = p.T @ t ; e = exp(S^T/BOOST - 3) ----
            e_sb = e_pool.tile([P, KT, HW], F8, tag="e")
            es_sb = esum_pool.tile([P, HW], F32, tag="es")
            es_bf = esum_pool.tile([P, HW], BF16, tag="esb", name=f"es_bf{s}")

            def s_mtile(m):
                ps_s = psum_mm.tile([P, HW], F32, tag="mm", name=f"ps_s{s}_{m}")
                for nq in range(NQ):
                    nc.tensor.matmul(
                        ps_s[:, nq * 512:(nq + 1) * 512],
                        tp_sb[:, 2:4, m * P:(m + 1) * P],
                        tp_sb[:, 0:2, nq * 512:nq * 512 + 512],
                        start=True,
                        stop=True,
                        perf_mode=DR,
                    )
                nc.scalar.activation(
                    e_sb[:, m, :], ps_s[:], AF.Exp,
                    bias=ebias_sb[:], scale=float(EXP_SCALE),
                )
                # running Z sum on DVE (fp32-internal, exact); the last add
                # emits bf16 so Zb can be a 1-cyc/row bf16 matmul
                if m == 1:
                    nc.vector.tensor_add(es_sb[:], e_sb[:, 0, :], e_sb[:, 1, :])
                elif m == KT - 1:
                    nc.vector.tensor_add(es_bf[:], es_sb[:], e_sb[:, m, :])
                elif m > 1:
                    nc.vector.tensor_add(es_sb[:], es_sb[:], e_sb[:, m, :])

            if s == 0:
                # sample 0: gt after S -- the wfu DMA rides behind x chunks on
                # the gpsimd ring and lands ~10us in; S gives it compute cover
                for m in range(KT):
                    s_mtile(m)
                for m in range(KT):
                    gt_mtile(m)
            else:
                # steady state: interleave gt between S tiles so PE stays busy
                # while ACT chews exps (730ns/tile vs 482ns/tile S rate)
                for m in range(KT):
                    s_mtile(m)
                    gt_mtile(m)

            # ---- y = G'T.T @ e : [C, HW] unnormalized attn+conv ----
            def y_mmtile(m):
                ps_y = psum_mm.tile([P, HW], F32, tag="mm", name=f"ps_y{s}_{m}")
                for kp in range(KT // 2):
                    for nq in range(NQ):
                        nc.tensor.matmul(
                            ps_y[:, nq * 512:(nq + 1) * 512],
                            gt_sb[:, 2 * kp:2 * kp + 2, m * P:(m + 1) * P],
                            e_sb[:, 2 * kp:2 * kp + 2, nq * 512:nq * 512 + 512],
                            start=(kp == 0),
                            stop=(kp == KT // 2 - 1),
                            perf_mode=DR,
                        )
                return ps_y

            ps_ys = [y_mmtile(0), y_mmtile(1)]

            # partition-sum + broadcast of Z: Zb = (16*ones).T @ es_bf
            ps_zbt = psum_mm.tile([P, HW], F32, tag="mm", name=f"ps_zb{s}")
            ps_zb, ps_zb2 = ps_zbt[:, 0:512], ps_zbt[:, 512:HW]
            for nq, pz in enumerate((ps_zb, ps_zb2)):
                nc.tensor.matmul(
                    pz[:],
                    ones_bf[:],
                    es_bf[:, nq * 512:(nq + 1) * 512],
                    start=True,
                    stop=True,
                )
            rzb_sb = rz_pool.tile([P, HW], F32, tag="rz")
            nc.vector.reciprocal_approx_fast(out=rzb_sb[:, 0:512], in_=ps_zb[:])
            nc.vector.reciprocal_approx_fast(out=rzb_sb[:, 512:HW], in_=ps_zb2[:])

            ps_ys += [y_mmtile(2), y_mmtile(3)]

            # final combine in 512-halves: mul (psum, must be DVE) then the
            # residual add on GpSimd for the last sample (its chain is the
            # kernel tail and GpSimd is idle there)
            for m in range(CT):
                t1 = fin_pool.tile([P, HW], F32, tag="fin", name=f"t1_{s}_{m}")
                o_sb = out_pool.tile([P, HW], F32, tag="o", name=f"o_{s}_{m}")
                add_eng = nc.gpsimd if s == NS - 1 else nc.vector
                for h in range(2):
                    hs = slice(h * 512, (h + 1) * 512)
                    nc.vector.tensor_mul(t1[:, hs], ps_ys[m][:, hs], rzb_sb[:, hs])
                    add_eng.tensor_add(
                        o_sb[:, hs], t1[:, hs],
                        xres_sb[:, m, h * 512:h * 512 + 512],
                    )
                    nc.sync.dma_start(
                        out_d[s].rearrange("(t p) f -> t p f", p=P)[m][:, hs],
                        o_sb[:, hs],
                    )


_CACHE = {}


def _build():
    if "nc" not in _CACHE:
        nc = bacc.Bacc("TRN2", target_bir_lowering=False, debug=False)
        with tile.TileContext(nc) as tc:
            _emit(tc)
        nc.compile()
        _CACHE["nc"] = nc
    return _CACHE["nc"]


def _prep_in_maps(x, W_theta, b_theta, W_phi, b_phi, W_fuse, b_fuse):
    bf = ml_dtypes.bfloat16
    f8 = ml_dtypes.float8_e4m3
    xf = np.ascontiguousarray(x.reshape(N, C, HW).astype(np.float32))
    x_f8 = xf.astype(f8)
    x_res = (xf + b_fuse.astype(np.float32)[None, :, None]).astype(bf)
    wcat_t = np.ascontiguousarray(
        np.concatenate([W_theta.astype(np.float32) * TP_BOOST,
                        W_phi.astype(np.float32) * TP_BOOST], axis=0).T
    ).astype(f8)
    b_cat = np.concatenate([b_theta.astype(np.float32) * TP_BOOST,
                            b_phi.astype(np.float32) * TP_BOOST]).reshape(2 * D, 1)
    wfu_t = np.ascontiguousarray(
        W_fuse.astype(np.float32).T * FU_BOOST
    ).astype(f8)

    in_maps = []
    for c in range(NCORES):
        sl = slice(c * NS, (c + 1) * NS)
        in_maps.append({
            "x_f8": np.ascontiguousarray(x_f8[sl]),
            "x_res": np.ascontiguousarray(x_res[sl]),
            "wcat_t": wcat_t,
            "b_cat": b_cat.astype(np.float32),
            "wfu_t": wfu_t,
        })
    return in_maps


def _run(inputs, trace=False, **kw):
    nc = _build()
    in_maps = _prep_in_maps(**inputs)
    res = bass_utils.run_bass_kernel_spmd(
        nc, in_maps, core_ids=list(range(NCORES)), trace=trace, **kw
    )
    out = np.concatenate([res.results[c]["out"] for c in range(NCORES)], axis=0)
    return out.reshape(N, C, H, W).astype(np.float32), res


def kernel(**inputs):
    inputs = {k: np.asarray(v) for k, v in inputs.items()}
    out, _ = _run(inputs, trace=False)
    return out
